# revision 7
# baseline (speedup 1.0000x reference)
"""Trainium2 Bass kernel for fused LN + QKV + QK-LN + RoPE + block-masked
attention + out-projection (nn_MultiHeadAttention_7103875908186).

Sharding: data-parallel over batch (2) x sequence-parallel over queries (4)
= 8 cores.  Each core owns 512 contiguous queries of one batch element and
receives a "key slab": the minimal contiguous seq_id-segment range covering
its queries, rolled so the 512 query rows sit at slab rows [0, 512), padded
to a common width Wk (SPMD uniformity).  The block mask (seq_id equality)
makes attention segment-local, so only the slab's keys can have nonzero
weight; padded/foreign keys are killed by a host-precomputed multiplicative
equality mask applied after exp().  Softmax is computed without max
subtraction (scores are O(6) after QK layernorm, so exp() cannot overflow)
and the denominator comes from an extra ones-column appended to V.
"""

import os
import sys

for _p in ("/opt/trn_rl_repo", os.path.expanduser("~/.axon_site/_ro/trn_rl_repo")):
    if os.path.isdir(_p) and _p not in sys.path:
        sys.path.insert(0, _p)

from contextlib import ExitStack

import ml_dtypes
import numpy as np

import concourse.bass as bass
import concourse.mybir as mybir
import concourse.tile as tile
from concourse import bacc
from concourse.bass_utils import run_bass_kernel_spmd
from concourse.masks import make_identity

B, L, D, H, DH = 2, 2048, 1536, 24, 64
EPS = 1e-5
ROPE_BASE = 10000.0
NCORES = 8
SHARDS = 4
NQ = L // SHARDS          # 512 queries per core
QT = NQ // 128            # 4 query tiles
FD = D // 128             # 12 feature blocks of 128
BF16 = ml_dtypes.bfloat16

f32 = mybir.dt.float32
bf16 = mybir.dt.bfloat16


# --------------------------------------------------------------------------
# device program
# --------------------------------------------------------------------------

def build_program(Wk: int, with_bias: bool):
    """Build the SPMD Bass program for key-slab width Wk (multiple of 128)."""
    T = Wk // 128
    nc = bacc.Bacc("TRN2", target_bir_lowering=False, num_devices=NCORES,
                   enable_asserts=False)

    xs = nc.dram_tensor("xs", [Wk, D], f32, kind="ExternalInput")
    wt = nc.dram_tensor("wt", [D, 3 * D], bf16, kind="ExternalInput")
    wot = nc.dram_tensor("wot", [D, D], bf16, kind="ExternalInput")
    cq = nc.dram_tensor("cq", [NQ, D], bf16, kind="ExternalInput")
    sq = nc.dram_tensor("sq", [NQ, D], bf16, kind="ExternalInput")
    ck = nc.dram_tensor("ck", [Wk, D], bf16, kind="ExternalInput")
    sk = nc.dram_tensor("sk", [Wk, D], bf16, kind="ExternalInput")
    em = nc.dram_tensor("em", [Wk, NQ], bf16, kind="ExternalInput")
    if with_bias:
        bq = nc.dram_tensor("bq", [1, 3 * D], f32, kind="ExternalInput")
    out = nc.dram_tensor("out", [NQ, D], f32, kind="ExternalOutput")

    wt_r = wt[:, :].rearrange("(dc p) f -> p dc f", p=128)      # [128, 12, 4608]
    wot_r = wot[:, :].rearrange("(fb p) e -> p fb e", p=128)    # [128, 12, 1536]

    with tile.TileContext(nc) as tc, ExitStack() as ctx:
        # ---- pools ------------------------------------------------------
        ps_tr = ctx.enter_context(tc.tile_pool(name="ps_tr", bufs=2, space="PSUM"))
        ps_mm = ctx.enter_context(tc.tile_pool(name="ps_mm", bufs=2, space="PSUM"))
        ps_s = ctx.enter_context(tc.tile_pool(name="ps_s", bufs=2, space="PSUM"))
        ps_ctx = ctx.enter_context(tc.tile_pool(name="ps_ctx", bufs=2, space="PSUM"))

        px = ctx.enter_context(tc.tile_pool(name="px", bufs=2))       # x stream
        pxn = ctx.enter_context(tc.tile_pool(name="pxn", bufs=2))     # x normed
        pxt = ctx.enter_context(tc.tile_pool(name="pxt", bufs=4))     # x~T per tile
        pw = ctx.enter_context(tc.tile_pool(name="pw", bufs=2))       # weight chunks
        pst = ctx.enter_context(tc.tile_pool(name="pst", bufs=6))     # stats / small
        pqk = ctx.enter_context(tc.tile_pool(name="pqk", bufs=4))     # q/k staging
        prot = ctx.enter_context(tc.tile_pool(name="prot", bufs=2))   # rotary tmp
        ptab = ctx.enter_context(tc.tile_pool(name="ptab", bufs=2))   # cos/sin
        pp = ctx.enter_context(tc.tile_pool(name="pp", bufs=3))       # P tiles
        pout = ctx.enter_context(tc.tile_pool(name="pout", bufs=2))   # out staging
        pden = ctx.enter_context(tc.tile_pool(name="pden", bufs=2))   # denominators

        # ---- persistent tiles -------------------------------------------
        pers = ctx.enter_context(tc.tile_pool(name="pers", bufs=1))
        id_bf = pers.tile([128, 128], bf16, name="id_bf")
        make_identity(nc, id_bf)
        eps_t = pers.tile([128, 1], f32, name="eps_t")
        nc.vector.memset(eps_t, EPS)

        kT = []   # 12 tiles [128, Wk] bf16, feature-major K (2 heads each)
        qT = []   # 12 tiles [128, NQ] bf16
        for fb in range(FD):
            kT.append(pers.tile([128, Wk], bf16, name=f"kT{fb}"))
            qT.append(pers.tile([128, NQ], bf16, name=f"qT{fb}"))
        v_aug = pers.tile([128, T, H, DH + 1], bf16, name="v_aug")
        ctxT = pers.tile([128, FD, NQ], bf16, name="ctxT")

        if with_bias:
            bias_t = pers.tile([128, 3 * D], f32, name="bias_t")
            bq_ap = bq[:, :]
            nc.sync.dma_start(out=bias_t, in_=bass.AP(
                tensor=bq_ap.tensor, offset=bq_ap.offset,
                ap=[[0, 128]] + list(bq_ap.ap[1:])))

        xT = [None] * T   # per-tile feature-major x~T

        def load_and_norm_x(t):
            """input LN: x~ = x * rsqrt(var+eps); mean removal folded into wt."""
            xa = px.tile([128, D], f32, name="xa")
            nc.sync.dma_start(out=xa, in_=xs[t * 128:(t + 1) * 128, :])
            st = pst.tile([128, 3, 6], f32, name="st_x")
            for i in range(3):
                nc.vector.bn_stats(out=st[:, i, :], in_=xa[:, i * 512:(i + 1) * 512])
            mv = pst.tile([128, 2], f32, name="mv_x")
            nc.vector.bn_aggr(out=mv, in_=st)
            sd = pst.tile([128, 1], f32, name="sd_x")
            nc.scalar.activation(sd, mv[:, 1:2], mybir.ActivationFunctionType.Sqrt,
                                 bias=eps_t)
            rr = pst.tile([128, 1], f32, name="rr_x")
            nc.vector.reciprocal(rr, sd)
            xn = pxn.tile([128, D], bf16, name="xn")
            nc.vector.tensor_scalar_mul(xn, xa, rr)
            # transpose to feature-major
            xt = pxt.tile([128, D], bf16, name="xt")
            for db in range(FD):
                pt_ = ps_tr.tile([128, 128], bf16, name="pt_tr")
                nc.tensor.transpose(pt_, xn[:, db * 128:(db + 1) * 128], id_bf)
                nc.any.tensor_copy(xt[:, db * 128:(db + 1) * 128], pt_)
            xT[t] = xt

        def qkv_chunk(fc, ts_list, stats, stage):
            """one 512-wide feature chunk of the qkv matmul for tiles ts_list.
            stats[t] collects bn_stats; stage[t] gets the bf16 copy."""
            wtile = pw.tile([128, FD, 512], bf16, name="wtile")
            nc.sync.dma_start(out=wtile, in_=wt_r[:, :, fc * 512:(fc + 1) * 512])
            kind = fc // 3            # 0=q, 1=k, 2=v
            sub = fc % 3
            for t in ts_list:
                pq = ps_mm.tile([128, 512], f32, name="pq_mm")
                for dc in range(FD):
                    nc.tensor.matmul(pq, xT[t][:, dc * 128:(dc + 1) * 128],
                                     wtile[:, dc, :],
                                     start=(dc == 0), stop=(dc == FD - 1))
                if kind == 2:
                    # v: straight into v_aug (8 heads per chunk), bf16
                    dst = v_aug[:, t, sub * 8:(sub + 1) * 8, 0:DH]
                    src = pq[:].rearrange("p (h d) -> p h d", h=8)
                    if with_bias:
                        ba = bias_t[:, (fc * 512):(fc + 1) * 512].rearrange(
                            "p (h d) -> p h d", h=8)
                        nc.any.tensor_add(dst, src, ba)
                    else:
                        nc.any.tensor_copy(dst, src)
                else:
                    nc.vector.bn_stats(out=stats[t][:, sub, :], in_=pq)
                    dst = stage[t][:, sub * 512:(sub + 1) * 512]
                    if with_bias:
                        nc.any.tensor_add(dst, pq,
                                          bias_t[:, fc * 512:(fc + 1) * 512])
                    else:
                        nc.any.tensor_copy(dst, pq)

        def ln_rope_transpose(t, stage_t, stats_t, cos_d, sin_d, dstT, ncols):
            """QK layernorm (mean+var over full D) + rotary + transpose into
            feature-major dstT tiles at column t*128."""
            if with_bias:
                # bias already added into stage; stats were taken pre-bias.
                # Recompute stats from staged values instead.
                st2 = pst.tile([128, 3, 6], f32, name="st2")
                for i in range(3):
                    nc.vector.bn_stats(out=st2[:, i, :],
                                       in_=stage_t[:, i * 512:(i + 1) * 512])
                stats_t = st2
            mv = pst.tile([128, 2], f32, name="mv_qk")
            nc.vector.bn_aggr(out=mv, in_=stats_t)
            sd = pst.tile([128, 1], f32, name="sd_qk")
            nc.scalar.activation(sd, mv[:, 1:2], mybir.ActivationFunctionType.Sqrt,
                                 bias=eps_t)
            rr = pst.tile([128, 1], f32, name="rr_qk")
            nc.vector.reciprocal(rr, sd)
            qh = prot.tile([128, H, 2, 32], bf16, name="qh")
            nc.vector.tensor_scalar(qh[:].rearrange("p h s j -> p (h s j)"),
                                    stage_t, mv[:, 0:1], rr,
                                    op0=mybir.AluOpType.subtract,
                                    op1=mybir.AluOpType.mult)
            cost = ptab.tile([128, D], bf16, name="cost")
            nc.sync.dma_start(out=cost, in_=cos_d[t * 128:(t + 1) * 128, :])
            sint = ptab.tile([128, H, 2, 32], bf16, name="sint")
            nc.sync.dma_start(out=sint[:].rearrange("p h s j -> p (h s j)"),
                              in_=sin_d[t * 128:(t + 1) * 128, :])
            qr = prot.tile([128, H, 2, 32], bf16, name="qr")
            nc.vector.tensor_mul(qr[:].rearrange("p h s j -> p (h s j)"),
                                 qh[:].rearrange("p h s j -> p (h s j)"), cost)
            rb = prot.tile([128, H, 2, 32], bf16, name="rb", bufs=1)
            nc.vector.tensor_mul(rb[:, :, 0, :], qh[:, :, 1, :], sint[:, :, 0, :])
            nc.vector.tensor_mul(rb[:, :, 1, :], qh[:, :, 0, :], sint[:, :, 1, :])
            nc.vector.tensor_add(qr[:].rearrange("p h s j -> p (h s j)"),
                                 qr[:].rearrange("p h s j -> p (h s j)"),
                                 rb[:].rearrange("p h s j -> p (h s j)"))
            qr_flat = qr[:].rearrange("p h s j -> p (h s j)")
            for fb in range(FD):
                pt_ = ps_tr.tile([128, 128], bf16, name="pt_tr")
                nc.tensor.transpose(pt_, qr_flat[:, fb * 128:(fb + 1) * 128], id_bf)
                nc.any.tensor_copy(dstT[fb][:, t * 128:(t + 1) * 128], pt_)

        # ================= phase 1: LN + QKV + QK-LN + RoPE ===============
        halves = [list(range(0, min(QT, T)))]
        if T > QT:
            halves.append(list(range(QT, T)))
        for hi, ts_list in enumerate(halves):
            for t in ts_list:
                load_and_norm_x(t)
            k_stats = {}
            k_stage = {}
            for t in ts_list:
                k_stats[t] = pst.tile([128, 3, 6], f32, name="st_k", bufs=QT + 1)
                k_stage[t] = pqk.tile([128, D], bf16, name="ksb")
            for fc in (3, 4, 5):
                qkv_chunk(fc, ts_list, k_stats, k_stage)
            for t in ts_list:
                ln_rope_transpose(t, k_stage[t], k_stats[t], ck, sk, kT, Wk)
            for fc in (6, 7, 8):
                qkv_chunk(fc, ts_list, None, None)
            nc.vector.memset(
                v_aug[:, ts_list[0]:ts_list[-1] + 1, :, DH:DH + 1], 1.0)
            if hi == 0:
                q_stats = {}
                q_stage = {}
                for t in ts_list[:QT]:
                    q_stats[t] = pst.tile([128, 3, 6], f32, name="st_q", bufs=QT + 1)
                    q_stage[t] = pqk.tile([128, D], bf16, name="qsb")
                for fc in (0, 1, 2):
                    qkv_chunk(fc, ts_list[:QT], q_stats, q_stage)
                for t in ts_list[:QT]:
                    ln_rope_transpose(t, q_stage[t], q_stats[t], cq, sq, qT, NQ)

        # ================= phase 2: attention =============================
        emt_all = pers.tile([128, T, NQ], bf16, name="emt_all")
        nc.sync.dma_start(
            out=emt_all,
            in_=em[:, :].rearrange("(kc p) q -> p kc q", p=128))
        emt = [emt_all[:, kc, :] for kc in range(T)]
        for h in range(H):
            fb = h // 2
            ro = (h % 2) * 64
            pc = ps_ctx.tile([DH + 1, NQ], f32, name="pc_ctx")
            for kc in range(T):
                ps = ps_s.tile([128, NQ], f32, name="ps_s")
                nc.tensor.matmul(ps, kT[fb][ro:ro + 64, kc * 128:(kc + 1) * 128],
                                 qT[fb][ro:ro + 64, :], start=True, stop=True)
                pe_ = pp.tile([128, NQ], bf16, name="pe_exp")
                nc.scalar.activation(pe_, ps, mybir.ActivationFunctionType.Exp,
                                     scale=float(1.0 / np.sqrt(DH)))
                pm = pp.tile([128, NQ], bf16, name="pm_mask")
                nc.vector.tensor_mul(pm, pe_, emt[kc])
                nc.tensor.matmul(pc, v_aug[:, kc, h, :], pm,
                                 start=(kc == 0), stop=(kc == T - 1))
            rden = pden.tile([1, NQ], f32, name="rden")
            nc.vector.reciprocal(rden, pc[DH:DH + 1, :])
            rdb = pden.tile([64, NQ], f32, name="rdb")
            nc.gpsimd.partition_broadcast(rdb, rden)
            nc.vector.tensor_mul(ctxT[ro:ro + 64, fb, :], pc[0:DH, :], rdb)

        # ================= phase 3: out projection ========================
        for ec in range(3):
            wo_t = pw.tile([128, FD, 512], bf16, name="wo_t", tag="wtile")
            nc.sync.dma_start(out=wo_t, in_=wot_r[:, :, ec * 512:(ec + 1) * 512])
            for qt in range(QT):
                po = ps_mm.tile([128, 512], f32, name="pq_mm")
                for fb in range(FD):
                    nc.tensor.matmul(po, ctxT[:, fb, qt * 128:(qt + 1) * 128],
                                     wo_t[:, fb, :],
                                     start=(fb == 0), stop=(fb == FD - 1))
                osb = pout.tile([128, 512], f32, name="osb")
                nc.any.tensor_copy(osb, po)
                nc.sync.dma_start(
                    out=out[qt * 128:(qt + 1) * 128, ec * 512:(ec + 1) * 512],
                    in_=osb)

    nc.compile()
    return nc


# --------------------------------------------------------------------------
# host-side preparation
# --------------------------------------------------------------------------

def host_prep(inputs):
    x = np.asarray(inputs["x"], np.float32)
    seq = np.asarray(inputs["seq_id"]).astype(np.int64)
    ln_w = np.asarray(inputs["ln_w"], np.float32)
    ln_b = np.asarray(inputs["ln_b"], np.float32)
    w_qkv = np.asarray(inputs["w_qkv"], np.float32)
    q_ln_w = np.asarray(inputs["q_ln_w"], np.float32)
    k_ln_w = np.asarray(inputs["k_ln_w"], np.float32)
    w_out = np.asarray(inputs["w_out"], np.float32)

    with_bias = bool(np.any(ln_b != 0.0))

    # fold ln_w and the input-LN mean into the QKV weight
    Wp = w_qkv * ln_w[None, :]
    Wpp = Wp - Wp.sum(1, keepdims=True) / D
    wt_host = np.ascontiguousarray(Wpp.T).astype(BF16)          # [D, 3D]
    wot_host = np.ascontiguousarray(w_out.T).astype(BF16)       # [D, D]
    bq_host = (w_qkv @ ln_b).astype(np.float32)[None, :]        # [1, 3D]

    # rope tables (64-wide, halves equal)
    inv = (1.0 / ROPE_BASE ** (np.arange(0, DH, 2, dtype=np.float64) / DH))

    def tables(pos, w):
        ang = pos[:, None].astype(np.float64) * inv[None, :]    # [N, 32]
        c64 = np.concatenate([np.cos(ang), np.cos(ang)], 1)     # [N, 64]
        s64 = np.concatenate([np.sin(ang), np.sin(ang)], 1)
        sign = np.concatenate([-np.ones(32), np.ones(32)])
        cos_e = np.tile(c64, (1, H)) * w[None, :]
        w_swap = w.reshape(H, 2, 32)[:, ::-1, :].reshape(-1)
        sin_e = np.tile(s64 * sign[None, :], (1, H)) * w_swap[None, :]
        return cos_e.astype(BF16), sin_e.astype(BF16)

    # per-core slabs
    ranges = []
    for c in range(NCORES):
        b, s = c // SHARDS, c % SHARDS
        q0 = s * NQ
        sq_ = seq[b]
        k0 = int(np.searchsorted(sq_, sq_[q0], side="left"))
        k1 = int(np.searchsorted(sq_, sq_[q0 + NQ - 1], side="right"))
        ranges.append((b, q0, k0, k1))
    wk_need = max(k1 - k0 for _, _, k0, k1 in ranges)
    Wk = max(((wk_need + 127) // 128) * 128, NQ + 128)
    Wk = min(Wk, L)

    in_maps = []
    for c in range(NCORES):
        b, q0, k0, k1 = ranges[c]
        order = (list(range(q0, q0 + NQ)) + list(range(k0, q0))
                 + list(range(q0 + NQ, k1)))
        order = order[: Wk]   # safety (cannot exceed by construction)
        npad = Wk - len(order)
        idx = np.array(order, np.int64)

        xs_c = np.zeros((Wk, D), np.float32)
        xs_c[: len(idx)] = x[b, idx]
        kid = np.full((Wk,), -1, np.int64)
        kid[: len(idx)] = seq[b, idx]
        qid = seq[b, q0:q0 + NQ]

        pos_k = np.zeros((Wk,), np.int64)
        pos_k[: len(idx)] = idx
        cq_c, sq_c = tables(np.arange(q0, q0 + NQ), q_ln_w)
        ck_c, sk_c = tables(pos_k, k_ln_w)

        em_c = (kid[:, None] == qid[None, :]).astype(BF16)      # [Wk, NQ]

        m = {
            "xs": xs_c,
            "wt": wt_host,
            "wot": wot_host,
            "cq": cq_c, "sq": sq_c, "ck": ck_c, "sk": sk_c,
            "em": em_c,
        }
        if with_bias:
            m["bq"] = bq_host
        in_maps.append(m)
    return in_maps, Wk, with_bias, [r[:2] for r in ranges]


_prog_cache = {}


def get_program(Wk, with_bias):
    key = (Wk, with_bias)
    if key not in _prog_cache:
        _prog_cache[key] = build_program(Wk, with_bias)
    return _prog_cache[key]


def kernel(**inputs) -> np.ndarray:
    in_maps, Wk, with_bias, qinfo = host_prep(inputs)
    nc = get_program(Wk, with_bias)
    res = run_bass_kernel_spmd(nc, in_maps, list(range(NCORES)))
    out = np.empty((B, L, D), np.float32)
    for c in range(NCORES):
        b, q0 = qinfo[c]
        out[b, q0:q0 + NQ, :] = res.results[c]["out"]
    return out
